# revision 61
# baseline (speedup 1.0000x reference)
"""GNN message-passing aggregation kernel for Trainium2 (8 NeuronCores).

Math: y[n,o] = mean_k relu(mailbox[n,k,:] @ W1 + b1) @ W2 + b2
  mailbox [500000, 16, 7] fp32, W1 [7,40], W2 [40,3], b1 == 0 (asserted).

Host prep: X^T [112, nodes] bf16 per core (transpose + cast on host), so
the per-tile X^T [112,128] slice is the PE stationary directly -- no
on-device transpose, no dtype cast.

Per 128-node tile, z = X W1blk (640 cols) lands in PSUM via 2 matmuls.
Drain path by tile index mod 3:
  'd'  (i%3==0) DVE fused abs-reduce (sum_k relu z = (sum z + sum |z|)/2;
       the sum-z term goes to yps directly via a deferred 3-col PE
       matmul with tile_k(W1 @ W2)/32 weights)
  'g2' (else)   ACT relu-drain (pair-permuted weight cols) to bf16 + one
       GPSIMD halving add per tile + ONE fused DVE k-reduce per triple
Per tile-TRIPLE the three 40-col hsums (cols 0:40/40:80/80:120 of a
[128,128] hb buffer) are transposed by the PE's transpose mode into a
single-bank bf16 PSUM tile (a 32KB DMA transpose costs ~2.7us of SDMA
servicing at ~12.5us ring latency and was the global bottleneck), then
a DVE 2x bf16 copy evacuates it to SBUF, and ONE 9-col matmul applies
the zero-masked W2 stack (W2/32 rows 0:40 for the abs hsum, W2/16 rows
40:80 and 80:120 for the relu hsums) accumulating y into a GROUP-tile
single-buffered PSUM group; groups flush via DVE add (+b2) and one
contiguous-per-partition DMA out (the q-major node interleave makes
each flush 128 big descriptors -- a scattered flush poisons a DMA sem
lane for ~10us).  The group open and all direct-term matmuls are
deferred into the pending L2 stream so the single yps bank's
open/flush ordering is always legal; pending pre-drains near group
boundaries to keep the flush burst small.

Bulk x input chunks ride SWDGE (gpsimd) so their multi-us completion
latency stays off the HWDGE completion-sem lanes.  A warm-up burst of
dummy matmuls at t=0 trips the PE HAM clock gate to K=8/8 (2.4 GHz);
the steady state keeps all PE idle gaps well under the ~3.4us HAM
re-throttle window, so the warmth persists for the whole kernel.

Steady state is DVE-bound (~1.84us per triple: 824ns 'd' abs-reduce +
806ns fused red2 + 213ns ht evacuation), ~0.6us/tile.

Sharding: pure data parallel over nodes, 62500/core (padded 62592).
"""

import os
import numpy as np
import ml_dtypes
from contextlib import ExitStack

import concourse.bass as bass
import concourse.bacc as bacc
import concourse.tile as tile
import concourse.mybir as mybir
from concourse.bass import ds, ts
from concourse import bass_utils

F32 = mybir.dt.float32
BF16 = mybir.dt.bfloat16

N_FULL = 500000
K, F_IN, F_HID, F_OUT = 16, 7, 40, 3
FB = K * F_IN              # 112
N_CORES = 8
TILE_P = 128
NODES_REAL_PER_CORE = N_FULL // N_CORES          # 62500
CHUNK = 16                 # node-tiles per input DMA
GROUP = 144                # tiles per y-psum group (144*3 = 432 psum cols);
                           # must divide by 3 (triples) and 16 (CHUNK)

# drain-path pattern over tile index i % 3: slot 0 is 'd' (abs half,
# W2/32 + PE-direct), slots 1,2 are 'g2' (relu, W2/16).
N_PAT = 3

# L2 matmuls are emitted LAG triples behind their transpose so the PE
# FIFO never head-of-line blocks on an in-flight DMA transpose.
LAG = int(os.environ.get("KERNEL_LAG", "4"))

HB_BUFS = 12

# number of PE warm-up matmuls (432 cols each) before the main loop;
# ~4.3us of sustained PE busy is enough to trip the HAM SHORT window
WARMUP_MM = int(os.environ.get("KERNEL_WARMUP", "9"))

# w1b column layout: [0:640) k-inner z cols (col 16j+k), [640:643) direct
# (W1 @ W2)/32 cols, [643:1283) g2-permuted z cols (8j+k for k<8 at
# +643, 320+8j+k-8 for k>=8).
W1B_COLS = 1283


def build(nc, n_tiles, level=4):
    """Emit the full per-core program into nc.

    level: ablation. 4 = full; 3 = no L2/transpose (y = b2 only);
    2 = L1 + drains, no hb consumers; 1 = L1 matmuls only; 0 = DMA only.
    """
    n_nodes = n_tiles * TILE_P
    x = nc.dram_tensor("x", (FB, n_nodes), BF16, kind="ExternalInput")
    w1b = nc.dram_tensor("w1b", (FB, W1B_COLS), BF16, kind="ExternalInput")
    w2b = nc.dram_tensor("w2b", (TILE_P, 3 * F_OUT), BF16, kind="ExternalInput")
    b2rep = nc.dram_tensor("b2rep", (TILE_P, 3 * GROUP), F32, kind="ExternalInput")
    ident = nc.dram_tensor("ident", (TILE_P, TILE_P), BF16, kind="ExternalInput")
    y = nc.dram_tensor("y", (n_nodes, F_OUT), F32, kind="ExternalOutput")

    xap = x.ap()
    yap = y.ap()

    with ExitStack() as ctx:
        tc = ctx.enter_context(tile.TileContext(nc))
        const = ctx.enter_context(tc.tile_pool(name="const", bufs=1))
        xinp = ctx.enter_context(tc.tile_pool(name="xin", bufs=5))
        zp = ctx.enter_context(tc.tile_pool(name="z", bufs=3, space="PSUM"))
        habsp = ctx.enter_context(tc.tile_pool(name="habs", bufs=6))
        treep = ctx.enter_context(tc.tile_pool(name="tree", bufs=6))
        htp = ctx.enter_context(tc.tile_pool(name="ht", bufs=LAG + 2))
        ypsp = ctx.enter_context(tc.tile_pool(name="yps", bufs=1, space="PSUM"))
        htpsp = ctx.enter_context(tc.tile_pool(name="htps", bufs=1, space="PSUM"))
        ysbp = ctx.enter_context(tc.tile_pool(name="ysb", bufs=2))

        w1b_sb = const.tile([FB, W1B_COLS], BF16)
        nc.sync.dma_start(w1b_sb[:], w1b.ap())
        w2b_sb = const.tile([TILE_P, 3 * F_OUT], BF16)
        nc.sync.dma_start(w2b_sb[:], w2b.ap())
        b2rep_sb = const.tile([TILE_P, 3 * GROUP], F32, tag="b2rep")
        nc.sync.dma_start(b2rep_sb[:], b2rep.ap())
        ident_sb = const.tile([TILE_P, TILE_P], BF16, tag="ident")
        nc.sync.dma_start(ident_sb[:], ident.ap())
        zconst = const.tile([FB, 128], BF16)
        nc.gpsimd.memset(zconst[:], 0.0)

        # fixed hb buffers (stable tensor ids): cols 120:128 are zeroed
        # once and never rewritten; they feed zero rows of w2b after
        # transpose but must be finite, not garbage.
        hb_bufs = []
        for bi in range(HB_BUFS):
            hb0 = const.tile([TILE_P, 128], BF16, tag=f"hb{bi}")
            nc.gpsimd.memset(hb0[:, 120:128], 0.0)
            hb_bufs.append(hb0)

        # PE warm-up burst into the (not-yet-opened) yps bank: sustained
        # PE busy trips the HAM to K=8/8 (2.4 GHz).  Both operands are
        # zconst (memset, no DMA dependency), so the burst runs during
        # the initial w1b/x DMA wait and the PE is already warm when the
        # first real tile's data lands.  The first group's open matmul
        # overwrites the bank (start=True clears it).
        if WARMUP_MM:
            zw = ypsp.tile([TILE_P, 3 * GROUP], F32, tag="yps")
            for _ in range(2 * WARMUP_MM):
                nc.tensor.matmul(
                    zw[:, 0:128], zconst[:], zconst[:, 0:128],
                    start=True, stop=True,
                )

        yps_state = {"g": -1, "yps": None}
        pending = []

        def pop_pending():
            e = pending.pop(0)
            g = e["g"]
            if g != yps_state["g"]:
                # lazily open group g's single-buffered yps accumulation
                # bank; pops are ordered, so this always follows the
                # previous group's close+flush.
                yps = ypsp.tile([TILE_P, 3 * GROUP], F32, tag="yps")
                yps_state["g"] = g
                yps_state["yps"] = yps
                nc.tensor.matmul(
                    yps[:, 0 : 3 * GROUP], zconst[:],
                    w1b_sb[:, 0 : 3 * GROUP],
                    start=True, stop=False, skip_group_check=True,
                )
            yps = yps_state["yps"]
            # deferred direct-term matmul for the triple's 'd' tile
            nc.tensor.matmul(
                yps[:, ts(e["g_idx"] - 2, 3)],
                e["xts"], w1b_sb[:, 640:643],
                start=False, stop=False, skip_group_check=True,
            )
            nc.tensor.matmul(
                yps[:, e["cols"]], e["ht"][:, :], w2b_sb[:, e["wcols"]],
                start=False, stop=False, skip_group_check=True,
            )
            if e["flush"] is not None:
                flush_group(yps, e["flush"][0], e["flush"][1])

        def flush_group(g_yps, g_base, g_ntiles):
            """Drain yps group to SBUF (+b2) and DMA to DRAM."""
            ncols = 3 * g_ntiles
            ysb = ysbp.tile([TILE_P, 3 * GROUP], F32, tag="ysb")
            if g_yps is None:
                nc.vector.tensor_copy(ysb[:, 0:ncols], b2rep_sb[:, 0:ncols])
            else:
                # close the bank's accumulation group (adds zero, full-bank
                # WAW orders it after every per-tile accumulate).
                nc.tensor.matmul(
                    g_yps[:, 0 : 3 * GROUP], zconst[:],
                    w1b_sb[:, 0 : 3 * GROUP],
                    start=False, stop=True, skip_group_check=True,
                )
                nc.vector.tensor_add(
                    ysb[:, 0:ncols], g_yps[:, 0:ncols], b2rep_sb[:, 0:ncols]
                )
            # q-major node interleave within the group: partition q's
            # 3*g_ntiles output floats are ONE contiguous DRAM run, so
            # the flush is 128 big descriptors (fast completion; a slow
            # scattered flush poisons a transpose sem lane for ~10us).
            dst = yap[ds(g_base * TILE_P, g_ntiles * TILE_P), :].rearrange(
                "(q s) o -> q s o", s=g_ntiles
            )
            src_ap = ysb[:, 0 : 3 * g_ntiles].rearrange("q (s o) -> q s o", o=3)
            # sync ring is otherwise idle; keeps the issue cost off the
            # relu-saturated ACT queue (and the contiguous layout makes
            # completion fast, so no sem-lane pollution risk).
            nc.sync.dma_start(dst, src_ap)

        # Software-pipelined emission: per-engine queues are strict
        # program order, so each tile's dependent stages are emitted N
        # rounds later to keep every queue head runnable.
        zabs, habss, trs, dxts = {}, {}, {}, {}
        xins = {}
        n_chunks = (n_tiles + CHUNK - 1) // CHUNK
        PREFETCH = 3

        def emit_chunk_dma(c):
            if c >= n_chunks or c in xins:
                return
            nch = min(CHUNK, n_tiles - c * CHUNK)
            xin = xinp.tile([FB, CHUNK * TILE_P], BF16, tag="xin")
            # SWDGE (gpsimd) path: measured faster than either HWDGE ring
            # for the bulk input chunks (separate queue + sem mechanics).
            if nch == CHUNK:
                nc.gpsimd.dma_start(
                    xin[:, 0 : nch * TILE_P],
                    xap[:, ds(c * CHUNK * TILE_P, nch * TILE_P)],
                )
            else:
                nc.scalar.dma_start(
                    xin[:, 0 : nch * TILE_P],
                    xap[:, ds(c * CHUNK * TILE_P, nch * TILE_P)],
                )
            xins[c] = xin

        def stage_front(i):
            c, s = divmod(i, CHUNK)
            if i == 0:
                for pc in range(PREFETCH + 1):
                    emit_chunk_dma(pc)
            if level < 1:
                return
            path = 'd' if i % N_PAT == 0 else 'g2'
            wofs = 643 if path == 'g2' else 0
            xts = xins[c][:, ds(s * TILE_P, TILE_P)]
            if path == 'd':
                # the direct-term matmul for this tile is deferred into
                # the pending (L2) stream; keep its stationary AP.
                dxts[i // 3] = xts
            zab = zp.tile([TILE_P, 640], F32, tag="z")
            zabs[i] = zab
            nc.tensor.matmul(
                zab[:, 0:512], xts, w1b_sb[:, wofs : wofs + 512],
                start=True, stop=True,
            )
            nc.tensor.matmul(
                zab[:, 512:640],
                xts, w1b_sb[:, wofs + 512 : wofs + 640],
                start=True, stop=True,
            )

        def stage_drain1(i):
            if level < 2:
                zabs.pop(i, None)
                return
            path = 'd' if i % N_PAT == 0 else 'g2'
            zab = zabs.pop(i)
            hb = hb_bufs[(i // 3) % HB_BUFS]
            hcol = 40 * (i % 3)
            with nc.allow_low_precision("bf16 hsum is within tolerance"):
                if path == 'd':
                    nc.vector.tensor_reduce(
                        hb[:, hcol : hcol + 40],
                        zab[:, 0:640].rearrange("q (j k) -> q j k", k=K),
                        axis=mybir.AxisListType.X,
                        op=mybir.AluOpType.add,
                        apply_absolute_value=True,
                    )
                else:
                    habs = habsp.tile([TILE_P, 640], BF16, tag="habs")
                    habss[i] = habs
                    nc.scalar.activation(
                        habs[:], zab[:, 0:640],
                        mybir.ActivationFunctionType.Relu,
                    )

        def stage_fold(i):
            # per-tile fold into a shared pair tree tile; red2 then runs
            # ONCE per triple over both tiles' folds.
            if level < 2 or i % N_PAT == 0:
                return
            habs = habss.pop(i)
            if i % N_PAT == 1:
                tr = treep.tile([TILE_P, 640], BF16, tag="tree")
                trs[i // 3] = tr
                tofs = 0
            else:
                tr = trs[i // 3]
                tofs = 320
            nc.gpsimd.tensor_add(
                tr[:, tofs : tofs + 320], habs[:, 0:320], habs[:, 320:640]
            )

        def stage_red2(i):
            if level < 2 or i % N_PAT != 2:
                return
            hb = hb_bufs[(i // 3) % HB_BUFS]
            tr = trs.pop(i // 3)
            with nc.allow_low_precision("bf16 hsum is within tolerance"):
                # 3D form of the pair reduce: 80 groups (t,j) of k=8;
                # identical semantics to the 4D (t,j,k) version with a
                # flat 2D output.
                nc.vector.tensor_reduce(
                    hb[:, 40:120],
                    tr[:, 0:640].rearrange("q (g k) -> q g k", k=8),
                    axis=mybir.AxisListType.X,
                    op=mybir.AluOpType.add,
                )

        htpss = {}

        def stage_pexpose(i):
            # PE transpose-mode: htps = hb.T into the dedicated psum bank.
            # Bank is single-buffered; Tile serializes transpose(t) ->
            # copy(t) -> transpose(t+1), each link sub-300ns.
            if level < 4 or i % 3 != 2:
                return
            hb = hb_bufs[(i // 3) % HB_BUFS]
            htps = htpsp.tile([128, 128], BF16, tag="htps")
            htpss[i // 3] = htps
            nc.tensor.transpose(htps[:], hb[:], ident_sb[:])

        def stage_htcopy(i):
            # DVE 2x bf16 evacuation of the transposed block to SBUF.
            if level < 4 or i % 3 != 2:
                return
            g_idx = i % GROUP
            htps = htpss.pop(i // 3)
            ht = htp.tile([128, 128], BF16, tag="ht")
            nc.vector.tensor_copy(ht[:], htps[:])
            e = {
                "ht": ht,
                "g": i // GROUP,
                "g_idx": g_idx,
                "xts": dxts.pop(i // 3),
                "cols": ds(3 * (g_idx - 2), 9),
                "wcols": ds(0, 9),
                "flush": None,
            }
            if g_idx == GROUP - 1 or i == n_tiles - 1:
                e["flush"] = (i - g_idx, g_idx + 1)
            pending.append(e)
            if e["flush"] is not None:
                # eager drain at group end: yps is single-buffered, so the
                # next group's open matmul must come after this flush.
                while pending:
                    pop_pending()
            else:
                if len(pending) > LAG:
                    pop_pending()
                # gradual pre-drain in the last LAG+1 triples of the group
                # so the boundary burst stays tiny (a ~25-op PE burst
                # starves the drain engines and can flip the pipeline
                # into a slow convoy mode).
                if GROUP - 1 - g_idx <= 3 * (LAG + 1) and len(pending) > 1:
                    pop_pending()

        for r in range(n_tiles + 7):
            # ht evacuation first: the single htps bank's WAR chain
            # (copy(t) gates transpose(t+1)) must clear promptly.
            if 0 <= r - 6 < n_tiles:
                stage_htcopy(r - 6)
            if r < n_tiles:
                stage_front(r)
            if 0 <= r - 1 < n_tiles:
                stage_drain1(r - 1)
            if 0 <= r - 2 < n_tiles:
                stage_fold(r - 2)
            if 0 <= r - 3 < n_tiles:
                stage_red2(r - 3)
            if 0 <= r - 5 < n_tiles:
                stage_pexpose(r - 5)
            if r < n_tiles:
                c, s = divmod(r, CHUNK)
                if s == 0 and r > 0:
                    emit_chunk_dma(c + PREFETCH)
                    xins.pop(c - 2, None)

        while pending:
            pop_pending()


_CACHE = {}


def _get_prog():
    key = "prog"
    if key not in _CACHE:
        nc = bacc.Bacc(
            "TRN2", target_bir_lowering=False, debug=False,
            num_devices=N_CORES,
        )
        n_tiles = (NODES_REAL_PER_CORE + TILE_P - 1) // TILE_P  # 489
        build(nc, n_tiles, level=int(os.environ.get("KERNEL_LEVEL", "4")))
        nc.finalize()
        _CACHE[key] = (nc, n_tiles)
    return _CACHE[key]


def _host_weights(W1, b1, W2, b2):
    W1 = np.asarray(W1, np.float32)
    W2 = np.asarray(W2, np.float32)
    b2 = np.asarray(b2, np.float32)

    # k-inner z cols: col 16*j + k
    w1ki = np.zeros((K, F_IN, F_HID, K), np.float32)
    for k in range(K):
        w1ki[k, :, :, k] = W1
    w1ki = w1ki.reshape(FB, F_HID * K)
    # direct term: sum_k z_k @ W2/32 = X @ tile_k(W1 @ W2)/32
    wdir = np.tile(W1 @ W2 / 32.0, (K, 1))  # [112, 3]
    # g2-permuted cols: halves foldable by one contiguous add, result
    # j-major k-inner(8): col 8j+k for k<8, col 320+8j+(k-8) for k>=8.
    w1g2 = np.zeros((FB, 640), np.float32)
    for k in range(K):
        for j in range(F_HID):
            col = 8 * j + k if k < 8 else 320 + 8 * j + (k - 8)
            w1g2[7 * k : 7 * k + 7, col] = W1[:, j]
    w1b = np.concatenate([w1ki, wdir, w1g2], axis=1).astype(ml_dtypes.bfloat16)

    w2rows = np.zeros((TILE_P, 3 * F_OUT), np.float32)
    w2rows[0:F_HID, 0:F_OUT] = W2 / 32.0                      # abs hsum
    w2rows[40 : 40 + F_HID, F_OUT : 2 * F_OUT] = W2 / 16.0    # relu hsum B
    w2rows[80 : 80 + F_HID, 2 * F_OUT : 3 * F_OUT] = W2 / 16.0  # relu hsum C
    w2rows = w2rows.astype(ml_dtypes.bfloat16)
    b2rep = np.tile(b2, (TILE_P, GROUP)).astype(np.float32)
    return w1b, w2rows, b2rep


def kernel(mailbox, W1, b1, W2, b2, **_unused):
    mailbox = np.asarray(mailbox)
    assert mailbox.shape == (N_FULL, K, F_IN), mailbox.shape
    b1 = np.asarray(b1, np.float32)
    assert np.abs(b1).max() == 0.0, "kernel assumes b1 == 0"

    nc, n_tiles = _get_prog()
    n_nodes = n_tiles * TILE_P

    X = np.ascontiguousarray(mailbox, dtype=np.float32).reshape(N_FULL, FB)
    XT = np.ascontiguousarray(X.T.astype(ml_dtypes.bfloat16))  # [112, N]
    w1b, w2rows, b2rep = _host_weights(W1, b1, W2, np.asarray(b2, np.float32))
    identm = np.eye(TILE_P, dtype=ml_dtypes.bfloat16)

    # q-major node interleave per y-psum GROUP: node gbase*128 + q*ng + s
    # sits at (tile gbase+s, partition q), so each group flush writes one
    # contiguous 3*ng-float DRAM run per partition.
    ni = np.empty((n_tiles, TILE_P), np.int64)
    q = np.arange(TILE_P)[None, :]
    for gbase in range(0, n_tiles, GROUP):
        ng = min(GROUP, n_tiles - gbase)
        s = np.arange(ng)[:, None]
        ni[gbase : gbase + ng] = gbase * TILE_P + q * ng + s
    ni_flat = ni.reshape(-1)

    in_maps = []
    for c in range(N_CORES):
        xtp = np.zeros((FB, n_nodes), ml_dtypes.bfloat16)
        xtp[:, :NODES_REAL_PER_CORE] = XT[
            :, c * NODES_REAL_PER_CORE : (c + 1) * NODES_REAL_PER_CORE
        ]
        xc = np.ascontiguousarray(xtp[:, ni_flat])
        in_maps.append({"x": xc, "w1b": w1b, "w2b": w2rows, "b2rep": b2rep,
                        "ident": identm})

    trace = os.environ.get("KERNEL_TRACE", "0") == "1"
    kwargs = {}
    if os.environ.get("KERNEL_TRACE_DIR"):
        kwargs["tmpdir"] = os.environ["KERNEL_TRACE_DIR"]
    res = bass_utils.run_bass_kernel_spmd(
        nc, in_maps, core_ids=list(range(N_CORES)), trace=trace, **kwargs
    )
    _CACHE["last_exec_ns"] = res.exec_time_ns
    _CACHE["last_res"] = res
    out = np.concatenate(
        [res.results[c]["y"][:NODES_REAL_PER_CORE] for c in range(N_CORES)],
        axis=0,
    )
    return np.ascontiguousarray(out, dtype=np.float32)


# revision 62
# speedup vs baseline: 1.0052x; 1.0052x over previous
"""GNN message-passing aggregation kernel for Trainium2 (8 NeuronCores).

Math: y[n,o] = mean_k relu(mailbox[n,k,:] @ W1 + b1) @ W2 + b2
  mailbox [500000, 16, 7] fp32, W1 [7,40], W2 [40,3], b1 == 0 (asserted).

Host prep: X^T [112, nodes] bf16 per core (transpose + cast on host), so
the per-tile X^T [112,128] slice is the PE stationary directly -- no
on-device transpose, no dtype cast.

Per 128-node tile, z = X W1blk (640 cols) lands in PSUM via 2 matmuls.
Drain path by tile index mod 3:
  'd'  (i%3==0) DVE fused abs-reduce (sum_k relu z = (sum z + sum |z|)/2;
       the sum-z term goes to yps directly via a deferred 3-col PE
       matmul with tile_k(W1 @ W2)/32 weights)
  'g2' (else)   ACT relu-drain (pair-permuted weight cols) to bf16 + one
       GPSIMD halving add per tile + ONE fused DVE k-reduce per triple
Per tile-TRIPLE the three 40-col hsums (cols 0:40/40:80/80:120 of a
[128,128] hb buffer) are transposed by the PE's transpose mode into a
single-bank bf16 PSUM tile (a 32KB DMA transpose costs ~2.7us of SDMA
servicing at ~12.5us ring latency and was the global bottleneck), then
a DVE 2x bf16 copy evacuates it to SBUF, and ONE 9-col matmul applies
the zero-masked W2 stack (W2/32 rows 0:40 for the abs hsum, W2/16 rows
40:80 and 80:120 for the relu hsums) accumulating y into a GROUP-tile
single-buffered PSUM group; groups flush via DVE add (+b2) and one
contiguous-per-partition DMA out (the q-major node interleave makes
each flush 128 big descriptors -- a scattered flush poisons a DMA sem
lane for ~10us).  The group open and all direct-term matmuls are
deferred into the pending L2 stream so the single yps bank's
open/flush ordering is always legal; pending pre-drains near group
boundaries to keep the flush burst small.

Bulk x input chunks ride SWDGE (gpsimd) so their multi-us completion
latency stays off the HWDGE completion-sem lanes.  A warm-up burst of
dummy matmuls at t=0 trips the PE HAM clock gate to K=8/8 (2.4 GHz);
the steady state keeps all PE idle gaps well under the ~3.4us HAM
re-throttle window, so the warmth persists for the whole kernel.

Steady state is DVE-bound (~1.84us per triple: 824ns 'd' abs-reduce +
806ns fused red2 + 213ns ht evacuation), ~0.6us/tile.

Sharding: pure data parallel over nodes, 62500/core (padded 62592).
"""

import os
import numpy as np
import ml_dtypes
from contextlib import ExitStack

import concourse.bass as bass
import concourse.bacc as bacc
import concourse.tile as tile
import concourse.mybir as mybir
from concourse.bass import ds, ts
from concourse import bass_utils

F32 = mybir.dt.float32
BF16 = mybir.dt.bfloat16

N_FULL = 500000
K, F_IN, F_HID, F_OUT = 16, 7, 40, 3
FB = K * F_IN              # 112
N_CORES = 8
TILE_P = 128
NODES_REAL_PER_CORE = N_FULL // N_CORES          # 62500
CHUNK = 16                 # node-tiles per input DMA
GROUP = 144                # tiles per y-psum group (144*3 = 432 psum cols);
                           # must divide by 3 (triples) and 16 (CHUNK)

# drain-path pattern over tile index i % 3: slot 0 is 'd' (abs half,
# W2/32 + PE-direct), slots 1,2 are 'g2' (relu, W2/16).
N_PAT = 3

# L2 matmuls are emitted LAG triples behind their transpose so the PE
# FIFO never head-of-line blocks on an in-flight DMA transpose.
LAG = int(os.environ.get("KERNEL_LAG", "4"))

HB_BUFS = 12

# number of PE warm-up matmuls (432 cols each) before the main loop;
# ~4.3us of sustained PE busy is enough to trip the HAM SHORT window
WARMUP_MM = int(os.environ.get("KERNEL_WARMUP", "9"))

# w1b column layout: [0:640) k-inner z cols (col 16j+k), [640:643) direct
# (W1 @ W2)/32 cols, [643:1283) g2-permuted z cols (8j+k for k<8 at
# +643, 320+8j+k-8 for k>=8).
W1B_COLS = 1283


def build(nc, n_tiles, level=4):
    """Emit the full per-core program into nc.

    level: ablation. 4 = full; 3 = no L2/transpose (y = b2 only);
    2 = L1 + drains, no hb consumers; 1 = L1 matmuls only; 0 = DMA only.
    """
    n_nodes = n_tiles * TILE_P
    x = nc.dram_tensor("x", (FB, n_nodes), BF16, kind="ExternalInput")
    w1b = nc.dram_tensor("w1b", (FB, W1B_COLS), BF16, kind="ExternalInput")
    w2b = nc.dram_tensor("w2b", (TILE_P, 3 * F_OUT), BF16, kind="ExternalInput")
    b2rep = nc.dram_tensor("b2rep", (TILE_P, 3 * GROUP), F32, kind="ExternalInput")
    ident = nc.dram_tensor("ident", (TILE_P, TILE_P), BF16, kind="ExternalInput")
    y = nc.dram_tensor("y", (n_nodes, F_OUT), F32, kind="ExternalOutput")

    xap = x.ap()
    yap = y.ap()

    with ExitStack() as ctx:
        tc = ctx.enter_context(tile.TileContext(nc))
        const = ctx.enter_context(tc.tile_pool(name="const", bufs=1))
        xinp = ctx.enter_context(tc.tile_pool(name="xin", bufs=5))
        zp = ctx.enter_context(tc.tile_pool(name="z", bufs=3, space="PSUM"))
        habsp = ctx.enter_context(tc.tile_pool(name="habs", bufs=6))
        treep = ctx.enter_context(tc.tile_pool(name="tree", bufs=6))
        htp = ctx.enter_context(tc.tile_pool(name="ht", bufs=LAG + 2))
        ypsp = ctx.enter_context(tc.tile_pool(name="yps", bufs=1, space="PSUM"))
        htpsp = ctx.enter_context(tc.tile_pool(name="htps", bufs=1, space="PSUM"))
        ysbp = ctx.enter_context(tc.tile_pool(name="ysb", bufs=2))

        w1b_sb = const.tile([FB, W1B_COLS], BF16)
        nc.sync.dma_start(w1b_sb[:], w1b.ap())
        w2b_sb = const.tile([TILE_P, 3 * F_OUT], BF16)
        nc.sync.dma_start(w2b_sb[:], w2b.ap())
        b2rep_sb = const.tile([TILE_P, 3 * GROUP], F32, tag="b2rep")
        nc.sync.dma_start(b2rep_sb[:], b2rep.ap())
        ident_sb = const.tile([TILE_P, TILE_P], BF16, tag="ident")
        nc.sync.dma_start(ident_sb[:], ident.ap())
        zconst = const.tile([FB, 128], BF16)
        nc.gpsimd.memset(zconst[:], 0.0)

        # fixed hb buffers (stable tensor ids): cols 120:128 are zeroed
        # once and never rewritten; they feed zero rows of w2b after
        # transpose but must be finite, not garbage.
        hb_bufs = []
        for bi in range(HB_BUFS):
            hb0 = const.tile([TILE_P, 128], BF16, tag=f"hb{bi}")
            nc.gpsimd.memset(hb0[:, 120:128], 0.0)
            hb_bufs.append(hb0)

        # PE warm-up burst into the (not-yet-opened) yps bank: ~9
        # back-to-back 432-col matmuls = 4+ us of sustained PE busy ->
        # HAM K=8/8 (2.4 GHz).  The first real group's open matmul
        # overwrites the bank (start=True clears it).
        if WARMUP_MM:
            zw = ypsp.tile([TILE_P, 3 * GROUP], F32, tag="yps")
            for _ in range(WARMUP_MM):
                nc.tensor.matmul(
                    zw[:, 0 : 3 * GROUP], zconst[:],
                    w1b_sb[:, 0 : 3 * GROUP],
                    start=True, stop=True,
                )

        yps_state = {"g": -1, "yps": None}
        pending = []

        def pop_pending():
            e = pending.pop(0)
            g = e["g"]
            if g != yps_state["g"]:
                # lazily open group g's single-buffered yps accumulation
                # bank; pops are ordered, so this always follows the
                # previous group's close+flush.
                yps = ypsp.tile([TILE_P, 3 * GROUP], F32, tag="yps")
                yps_state["g"] = g
                yps_state["yps"] = yps
                nc.tensor.matmul(
                    yps[:, 0 : 3 * GROUP], zconst[:],
                    w1b_sb[:, 0 : 3 * GROUP],
                    start=True, stop=False, skip_group_check=True,
                )
            yps = yps_state["yps"]
            # deferred direct-term matmul for the triple's 'd' tile
            nc.tensor.matmul(
                yps[:, ts(e["g_idx"] - 2, 3)],
                e["xts"], w1b_sb[:, 640:643],
                start=False, stop=False, skip_group_check=True,
            )
            nc.tensor.matmul(
                yps[:, e["cols"]], e["ht"][:, :], w2b_sb[:, e["wcols"]],
                start=False, stop=False, skip_group_check=True,
            )
            if e["flush"] is not None:
                flush_group(yps, e["flush"][0], e["flush"][1])

        def flush_group(g_yps, g_base, g_ntiles):
            """Drain yps group to SBUF (+b2) and DMA to DRAM."""
            ncols = 3 * g_ntiles
            ysb = ysbp.tile([TILE_P, 3 * GROUP], F32, tag="ysb")
            if g_yps is None:
                nc.vector.tensor_copy(ysb[:, 0:ncols], b2rep_sb[:, 0:ncols])
            else:
                # close the bank's accumulation group (adds zero, full-bank
                # WAW orders it after every per-tile accumulate).
                nc.tensor.matmul(
                    g_yps[:, 0 : 3 * GROUP], zconst[:],
                    w1b_sb[:, 0 : 3 * GROUP],
                    start=False, stop=True, skip_group_check=True,
                )
                nc.vector.tensor_add(
                    ysb[:, 0:ncols], g_yps[:, 0:ncols], b2rep_sb[:, 0:ncols]
                )
            # q-major node interleave within the group: partition q's
            # 3*g_ntiles output floats are ONE contiguous DRAM run, so
            # the flush is 128 big descriptors (fast completion; a slow
            # scattered flush poisons a transpose sem lane for ~10us).
            dst = yap[ds(g_base * TILE_P, g_ntiles * TILE_P), :].rearrange(
                "(q s) o -> q s o", s=g_ntiles
            )
            src_ap = ysb[:, 0 : 3 * g_ntiles].rearrange("q (s o) -> q s o", o=3)
            # sync ring is otherwise idle; keeps the issue cost off the
            # relu-saturated ACT queue (and the contiguous layout makes
            # completion fast, so no sem-lane pollution risk).
            nc.sync.dma_start(dst, src_ap)

        # Software-pipelined emission: per-engine queues are strict
        # program order, so each tile's dependent stages are emitted N
        # rounds later to keep every queue head runnable.
        zabs, habss, trs, dxts = {}, {}, {}, {}
        xins = {}
        n_chunks = (n_tiles + CHUNK - 1) // CHUNK
        PREFETCH = 3

        def emit_chunk_dma(c):
            if c >= n_chunks or c in xins:
                return
            nch = min(CHUNK, n_tiles - c * CHUNK)
            xin = xinp.tile([FB, CHUNK * TILE_P], BF16, tag="xin")
            # SWDGE (gpsimd) path: measured faster than either HWDGE ring
            # for the bulk input chunks (separate queue + sem mechanics).
            if nch == CHUNK:
                nc.gpsimd.dma_start(
                    xin[:, 0 : nch * TILE_P],
                    xap[:, ds(c * CHUNK * TILE_P, nch * TILE_P)],
                )
            else:
                nc.scalar.dma_start(
                    xin[:, 0 : nch * TILE_P],
                    xap[:, ds(c * CHUNK * TILE_P, nch * TILE_P)],
                )
            xins[c] = xin

        def stage_front(i):
            c, s = divmod(i, CHUNK)
            if i == 0:
                for pc in range(PREFETCH + 1):
                    emit_chunk_dma(pc)
            if level < 1:
                return
            path = 'd' if i % N_PAT == 0 else 'g2'
            wofs = 643 if path == 'g2' else 0
            xts = xins[c][:, ds(s * TILE_P, TILE_P)]
            if path == 'd':
                # the direct-term matmul for this tile is deferred into
                # the pending (L2) stream; keep its stationary AP.
                dxts[i // 3] = xts
            zab = zp.tile([TILE_P, 640], F32, tag="z")
            zabs[i] = zab
            nc.tensor.matmul(
                zab[:, 0:512], xts, w1b_sb[:, wofs : wofs + 512],
                start=True, stop=True,
            )
            nc.tensor.matmul(
                zab[:, 512:640],
                xts, w1b_sb[:, wofs + 512 : wofs + 640],
                start=True, stop=True,
            )

        def stage_drain1(i):
            if level < 2:
                zabs.pop(i, None)
                return
            path = 'd' if i % N_PAT == 0 else 'g2'
            zab = zabs.pop(i)
            hb = hb_bufs[(i // 3) % HB_BUFS]
            hcol = 40 * (i % 3)
            with nc.allow_low_precision("bf16 hsum is within tolerance"):
                if path == 'd':
                    nc.vector.tensor_reduce(
                        hb[:, hcol : hcol + 40],
                        zab[:, 0:640].rearrange("q (j k) -> q j k", k=K),
                        axis=mybir.AxisListType.X,
                        op=mybir.AluOpType.add,
                        apply_absolute_value=True,
                    )
                else:
                    habs = habsp.tile([TILE_P, 640], BF16, tag="habs")
                    habss[i] = habs
                    nc.scalar.activation(
                        habs[:], zab[:, 0:640],
                        mybir.ActivationFunctionType.Relu,
                    )

        def stage_fold(i):
            # per-tile fold into a shared pair tree tile; red2 then runs
            # ONCE per triple over both tiles' folds.
            if level < 2 or i % N_PAT == 0:
                return
            habs = habss.pop(i)
            if i % N_PAT == 1:
                tr = treep.tile([TILE_P, 640], BF16, tag="tree")
                trs[i // 3] = tr
                tofs = 0
            else:
                tr = trs[i // 3]
                tofs = 320
            nc.gpsimd.tensor_add(
                tr[:, tofs : tofs + 320], habs[:, 0:320], habs[:, 320:640]
            )

        def stage_red2(i):
            if level < 2 or i % N_PAT != 2:
                return
            hb = hb_bufs[(i // 3) % HB_BUFS]
            tr = trs.pop(i // 3)
            with nc.allow_low_precision("bf16 hsum is within tolerance"):
                # 3D form of the pair reduce: 80 groups (t,j) of k=8;
                # identical semantics to the 4D (t,j,k) version with a
                # flat 2D output.
                nc.vector.tensor_reduce(
                    hb[:, 40:120],
                    tr[:, 0:640].rearrange("q (g k) -> q g k", k=8),
                    axis=mybir.AxisListType.X,
                    op=mybir.AluOpType.add,
                )

        htpss = {}

        def stage_pexpose(i):
            # PE transpose-mode: htps = hb.T into the dedicated psum bank.
            # Bank is single-buffered; Tile serializes transpose(t) ->
            # copy(t) -> transpose(t+1), each link sub-300ns.
            if level < 4 or i % 3 != 2:
                return
            hb = hb_bufs[(i // 3) % HB_BUFS]
            htps = htpsp.tile([128, 128], BF16, tag="htps")
            htpss[i // 3] = htps
            nc.tensor.transpose(htps[:], hb[:], ident_sb[:])

        def stage_htcopy(i):
            # DVE 2x bf16 evacuation of the transposed block to SBUF.
            if level < 4 or i % 3 != 2:
                return
            g_idx = i % GROUP
            htps = htpss.pop(i // 3)
            ht = htp.tile([128, 128], BF16, tag="ht")
            nc.vector.tensor_copy(ht[:], htps[:])
            e = {
                "ht": ht,
                "g": i // GROUP,
                "g_idx": g_idx,
                "xts": dxts.pop(i // 3),
                "cols": ds(3 * (g_idx - 2), 9),
                "wcols": ds(0, 9),
                "flush": None,
            }
            if g_idx == GROUP - 1 or i == n_tiles - 1:
                e["flush"] = (i - g_idx, g_idx + 1)
            pending.append(e)
            if e["flush"] is not None:
                # eager drain at group end: yps is single-buffered, so the
                # next group's open matmul must come after this flush.
                while pending:
                    pop_pending()
            else:
                if len(pending) > LAG:
                    pop_pending()
                # gradual pre-drain in the last LAG+1 triples of the group
                # so the boundary burst stays tiny (a ~25-op PE burst
                # starves the drain engines and can flip the pipeline
                # into a slow convoy mode).
                if GROUP - 1 - g_idx <= 3 * (LAG + 1) and len(pending) > 1:
                    pop_pending()

        for r in range(n_tiles + 7):
            # ht evacuation first: the single htps bank's WAR chain
            # (copy(t) gates transpose(t+1)) must clear promptly.
            if 0 <= r - 6 < n_tiles:
                stage_htcopy(r - 6)
            if r < n_tiles:
                stage_front(r)
            if 0 <= r - 1 < n_tiles:
                stage_drain1(r - 1)
            if 0 <= r - 2 < n_tiles:
                stage_fold(r - 2)
            if 0 <= r - 3 < n_tiles:
                stage_red2(r - 3)
            if 0 <= r - 5 < n_tiles:
                stage_pexpose(r - 5)
            if r < n_tiles:
                c, s = divmod(r, CHUNK)
                if s == 0 and r > 0:
                    emit_chunk_dma(c + PREFETCH)
                    xins.pop(c - 2, None)

        while pending:
            pop_pending()


_CACHE = {}


def _get_prog():
    key = "prog"
    if key not in _CACHE:
        nc = bacc.Bacc(
            "TRN2", target_bir_lowering=False, debug=False,
            num_devices=N_CORES,
        )
        n_tiles = (NODES_REAL_PER_CORE + TILE_P - 1) // TILE_P  # 489
        build(nc, n_tiles, level=int(os.environ.get("KERNEL_LEVEL", "4")))
        nc.finalize()
        _CACHE[key] = (nc, n_tiles)
    return _CACHE[key]


def _host_weights(W1, b1, W2, b2):
    W1 = np.asarray(W1, np.float32)
    W2 = np.asarray(W2, np.float32)
    b2 = np.asarray(b2, np.float32)

    # k-inner z cols: col 16*j + k
    w1ki = np.zeros((K, F_IN, F_HID, K), np.float32)
    for k in range(K):
        w1ki[k, :, :, k] = W1
    w1ki = w1ki.reshape(FB, F_HID * K)
    # direct term: sum_k z_k @ W2/32 = X @ tile_k(W1 @ W2)/32
    wdir = np.tile(W1 @ W2 / 32.0, (K, 1))  # [112, 3]
    # g2-permuted cols: halves foldable by one contiguous add, result
    # j-major k-inner(8): col 8j+k for k<8, col 320+8j+(k-8) for k>=8.
    w1g2 = np.zeros((FB, 640), np.float32)
    for k in range(K):
        for j in range(F_HID):
            col = 8 * j + k if k < 8 else 320 + 8 * j + (k - 8)
            w1g2[7 * k : 7 * k + 7, col] = W1[:, j]
    w1b = np.concatenate([w1ki, wdir, w1g2], axis=1).astype(ml_dtypes.bfloat16)

    w2rows = np.zeros((TILE_P, 3 * F_OUT), np.float32)
    w2rows[0:F_HID, 0:F_OUT] = W2 / 32.0                      # abs hsum
    w2rows[40 : 40 + F_HID, F_OUT : 2 * F_OUT] = W2 / 16.0    # relu hsum B
    w2rows[80 : 80 + F_HID, 2 * F_OUT : 3 * F_OUT] = W2 / 16.0  # relu hsum C
    w2rows = w2rows.astype(ml_dtypes.bfloat16)
    b2rep = np.tile(b2, (TILE_P, GROUP)).astype(np.float32)
    return w1b, w2rows, b2rep


def kernel(mailbox, W1, b1, W2, b2, **_unused):
    mailbox = np.asarray(mailbox)
    assert mailbox.shape == (N_FULL, K, F_IN), mailbox.shape
    b1 = np.asarray(b1, np.float32)
    assert np.abs(b1).max() == 0.0, "kernel assumes b1 == 0"

    nc, n_tiles = _get_prog()
    n_nodes = n_tiles * TILE_P

    X = np.ascontiguousarray(mailbox, dtype=np.float32).reshape(N_FULL, FB)
    XT = np.ascontiguousarray(X.T.astype(ml_dtypes.bfloat16))  # [112, N]
    w1b, w2rows, b2rep = _host_weights(W1, b1, W2, np.asarray(b2, np.float32))
    identm = np.eye(TILE_P, dtype=ml_dtypes.bfloat16)

    # q-major node interleave per y-psum GROUP: node gbase*128 + q*ng + s
    # sits at (tile gbase+s, partition q), so each group flush writes one
    # contiguous 3*ng-float DRAM run per partition.
    ni = np.empty((n_tiles, TILE_P), np.int64)
    q = np.arange(TILE_P)[None, :]
    for gbase in range(0, n_tiles, GROUP):
        ng = min(GROUP, n_tiles - gbase)
        s = np.arange(ng)[:, None]
        ni[gbase : gbase + ng] = gbase * TILE_P + q * ng + s
    ni_flat = ni.reshape(-1)

    in_maps = []
    for c in range(N_CORES):
        xtp = np.zeros((FB, n_nodes), ml_dtypes.bfloat16)
        xtp[:, :NODES_REAL_PER_CORE] = XT[
            :, c * NODES_REAL_PER_CORE : (c + 1) * NODES_REAL_PER_CORE
        ]
        xc = np.ascontiguousarray(xtp[:, ni_flat])
        in_maps.append({"x": xc, "w1b": w1b, "w2b": w2rows, "b2rep": b2rep,
                        "ident": identm})

    trace = os.environ.get("KERNEL_TRACE", "0") == "1"
    kwargs = {}
    if os.environ.get("KERNEL_TRACE_DIR"):
        kwargs["tmpdir"] = os.environ["KERNEL_TRACE_DIR"]
    res = bass_utils.run_bass_kernel_spmd(
        nc, in_maps, core_ids=list(range(N_CORES)), trace=trace, **kwargs
    )
    _CACHE["last_exec_ns"] = res.exec_time_ns
    _CACHE["last_res"] = res
    out = np.concatenate(
        [res.results[c]["y"][:NODES_REAL_PER_CORE] for c in range(N_CORES)],
        axis=0,
    )
    return np.ascontiguousarray(out, dtype=np.float32)
